# revision 45
# baseline (speedup 1.0000x reference)
"""Trainium2 Bass kernel for nn_DeformConv2d_3246995276085 (v2).

Structural insight (from v1): the reference feeds pixel-space coords into a
grid_sample expecting [-1,1] coords, so only an 11x11 corner of each image
contributes; feat is nonzero only at flat positions L in runs
[864*i, 864*i+99), and the final conv output only at rows {9i-1..9i+2}.

v2 redesign (latency-driven; each DMA hop costs ~2.4us in fixed overheads):
- Host folds alpha into the offset-conv weights (the blend is linear), and
  folds the 48*g+47.5 coordinate affine plus base-grid terms into extra
  contraction rows of the conv -> ONE 9-tap matmul set emits pixel coords
  IX||IY [66, 18] directly in PSUM.
- Gather indices (16-wrapped int16) and per-slot bilinear weights are built
  ON-CHIP with small select-matrix matmuls (host-precomputed 0/1 operands)
  instead of a DRAM streamout + readback round trip.
- ONE merged dma_gather (12 chunks of 128 slots; chunk = (y-row, stream), so
  chunk slot p = 9*j+d equals the feat run position k) fetches x row-pairs.
- Modulation conv runs with output replicated across 64 partitions (free);
  sigmoid(mod) is multiplied in during the PSUM->feat transposed copy.
- Weighted combine uses per-partition scalar ops (W4S[:,s,q] pointers).
"""

import functools

import numpy as np

ND = 9
C = 64
H = W = 96
NJ = 11          # j extent of corner region
NS = 6           # strip-rows (i values) per core
NM = 66          # corner pixels per core (NS * NJ)
NK = 67          # offset-conv contraction rows (64 ch + bias + i-map + j-map)
NCH = 12         # gather chunks (6 streams x {y0,y1})
NIDX = NCH * 128
XHROWS = 9606    # padded HWC image rows (98*98 + 2 spare)

DIRY = np.array([0, 0, 0, 1, 1, 1, -1, -1, -1], np.float32)
DIRX = np.array([0, 1, -1, 0, 1, -1, 0, 1, -1], np.float32)

# blobA (fp32, [NK, 756]): XW3 [NK,9,66] cols 0:594; WOFF2 [NK,9,18] 594:756
A_XW = 0
A_WOFF = 594
A_COLS = 756
# blobM (fp32, [66, 439]): MCW [66,9,6,8] 0:432; MS [66,6] 432:438; BMOD 438
M_MCW = 0
M_MS = 432
M_BMOD = 438
M_COLS = 439
# blobS (bf16, [66, 2880]): SELI [66,9,128] 0:1152; SELW [66,9,128]
#   1152:2304; WMODR [64,9,64] 2304:2880.  SELI rows repeat mod 16 so the
#   idx scatter emits all 128 partitions (gather hw reads 8 replicated
#   groups of 16).
S_SELI = 0
S_SELW = 1152
S_WMODR = 2304
S_COLS = 2880
# blobX (bf16, [64, 5346]): XMOD [64,9,6,99]
X_COLS = ND * NS * 99
# blobI (fp32, [128,128]) identity; blobW (bf16, [64,576]) final-conv weights


# ----------------------------------------------------------------- host prep

def _make_xhwcp(xb):
    """xb (64, 96, 96) -> zero-padded HWC (XHROWS, 64): row/col pad of 1,
    pixel (y, x) at slot (y+1)*98 + (x+1)."""
    out = np.zeros((XHROWS, C), np.float32)
    v = out[:9604].reshape(98, 98, C)
    v[1:97, 1:97, :] = xb.transpose(1, 2, 0)
    return out


@functools.lru_cache(maxsize=1)
def _shared_consts():
    """Input-independent select/mask blobs (as float64-safe numpy)."""
    # MCW[d, m, s, w] = (m//11==s) and ((9*(m%11)+d)//16 == w)
    mcw = np.zeros((ND, NM, NS, 8), np.float32)
    ms = np.zeros((NM, NS), np.float32)
    seli = np.zeros((ND, NM, 128), np.float32)
    selw = np.zeros((ND, NM, 128), np.float32)
    for m in range(NM):
        s, j = m // NJ, m % NJ
        ms[m, s] = 1.0
        for d in range(ND):
            p = 9 * j + d
            mcw[d, m, s, p // 16] = 1.0
            seli[d, m, (p % 16)::16] = 1.0
            selw[d, m, p] = 1.0
    return mcw, ms, seli, selw


def _make_core_inputs(x, w_off1, b_off1, w_off2, b_off2, w_mod, b_mod,
                      conv_weight, alpha, b, part):
    import ml_dtypes
    bf16 = ml_dtypes.bfloat16
    i0 = 6 * part
    xb = x[b]
    al = np.float32(alpha)

    weff = (al * w_off1 + (1 - al) * w_off2).astype(np.float32)   # (18,C,3,3)
    beff = (al * b_off1 + (1 - al) * b_off2).astype(np.float32)   # (18,)

    # blobA: XW3 (flat per-tap windows; lhsT needs a single free dim) + WOFF2
    blobA = np.zeros((NK, A_COLS), np.float32)
    xw = np.zeros((NK, ND, NM), np.float32)
    marr = np.arange(NM)
    irow = i0 + marr // NJ
    jcol = marr % NJ
    for t in range(9):
        dy, dx = t // 3 - 1, t % 3 - 1
        rr, cc2 = irow + dy, jcol + dx
        sel = (rr >= 0) & (rr < H) & (cc2 >= 0) & (cc2 < W)
        xw[0:64, t, sel] = xb[:, rr[sel], cc2[sel]]
    xw[64, 4, :] = 1.0
    xw[65, 4, :] = 48.0 * irow
    xw[66, 4, :] = 48.0 * jcol
    blobA[:, A_XW:A_XW + 594] = xw.reshape(NK, 594)
    woff = np.zeros((NK, ND, 18), np.float32)
    for t in range(9):
        dy, dx = t // 3, t % 3
        woff[0:64, t, 0:9] = 48.0 * weff[0:9, :, dy, dx].T
        woff[0:64, t, 9:18] = 48.0 * weff[9:18, :, dy, dx].T
    woff[64, 4, 0:9] = 48.0 * beff[0:9] + 48.0 * DIRY + 47.5
    woff[64, 4, 9:18] = 48.0 * beff[9:18] + 48.0 * DIRX + 47.5
    woff[65, 4, 0:9] = 1.0
    woff[66, 4, 9:18] = 1.0
    blobA[:, A_WOFF:A_WOFF + 162] = woff.reshape(NK, 162)

    # blobM: masks + BMOD
    mcw, msk, seli, selw = _shared_consts()
    blobM = np.zeros((NM, M_COLS), np.float32)
    blobM[:, M_MCW:M_MCW + 432] = mcw.transpose(1, 0, 2, 3).reshape(NM, 432)
    blobM[:, M_MS:M_MS + 6] = msk
    blobM[0:64, M_BMOD] = np.float32(b_mod[0])

    # blobS: SELI + SELW + WMODR
    blobS = np.zeros((NM, S_COLS), bf16)
    blobS[:, S_SELI:S_SELI + 1152] = seli.transpose(1, 0, 2).reshape(
        NM, 1152).astype(bf16)
    blobS[:, S_SELW:S_SELW + 1152] = selw.transpose(1, 0, 2).reshape(
        NM, 1152).astype(bf16)
    wmodr = np.zeros((NM, ND, 64), np.float32)
    for t in range(9):
        dy, dx = t // 3, t % 3
        wmodr[0:64, t, :] = w_mod[0, :, dy, dx][:, None]
    blobS[:, S_WMODR:S_WMODR + 576] = wmodr.reshape(NM, 576).astype(bf16)

    # blobX: XMOD[c, t, s, kk] = x at (9*(i0+s)+phi+dy, j2+dx), phi=kk>=96
    xmod = np.zeros((C, ND, NS, 99), np.float32)
    xp = np.zeros((H + 2, W + 2), np.float32)
    for t in range(9):
        dy, dx = t // 3 - 1, t % 3 - 1
        for s in range(NS):
            for phi, k0, kn in ((0, 0, 96), (1, 96, 3)):
                row = 9 * (i0 + s) + phi + dy
                if not (0 <= row < H):
                    continue
                c0 = dx
                # cols j2+dx for j2 in [0, kn): clip to [0, 96)
                j2 = np.arange(kn)
                cols = j2 + dx
                sel = (cols >= 0) & (cols < W)
                xmod[:, t, s, k0 + j2[sel]] = xb[:, row, cols[sel]]
    blobX = xmod.reshape(C, X_COLS).astype(bf16)

    blobI = np.eye(128, dtype=np.float32)

    # WCNV2A [(u*64+c), dyk, o] = w[o, c, dyk, u] (dx taps 0,1 folded into
    # contraction); WCNV2B [c, dyk, o] = w[o, c, dyk, 2]
    blobW = np.zeros((128, 384), np.float32)
    for dyk in range(3):
        for u in range(2):
            blobW[64 * u:64 * u + 64, 64 * dyk:64 * dyk + 64] = \
                conv_weight[:, :, dyk, u].T
        blobW[0:64, 192 + 64 * dyk:192 + 64 * dyk + 64] = \
            conv_weight[:, :, dyk, 2].T
    blobW = blobW.astype(bf16)

    return {
        "xh": _make_xhwcp(xb),
        "blobA": blobA,
        "blobM": blobM,
        "blobS": np.asarray(blobS),
        "blobX": np.asarray(blobX),
        "blobI": blobI,
        "blobW": np.asarray(blobW),
    }


# ------------------------------------------------------------- device kernel

def emit_kernel(tc, outs, ins):
    from contextlib import ExitStack

    import concourse.bass as bass
    from concourse import mybir

    ctx = ExitStack()

    dt = mybir.dt
    Alu = mybir.AluOpType
    Act = mybir.ActivationFunctionType
    nc = tc.nc
    f32 = dt.float32
    bf = dt.bfloat16

    xh = ins["xh"]
    strips_out = outs["strips_out"]

    consts = ctx.enter_context(tc.tile_pool(name="consts", bufs=1))
    work = ctx.enter_context(tc.tile_pool(name="work", bufs=1))
    loop_sb = ctx.enter_context(tc.tile_pool(name="loop_sb", bufs=3))
    psA = ctx.enter_context(tc.tile_pool(name="psA", bufs=1, space="PSUM"))
    psM = ctx.enter_context(tc.tile_pool(name="psM", bufs=1, space="PSUM"))
    psC = ctx.enter_context(tc.tile_pool(name="psC", bufs=1, space="PSUM"))
    psD = ctx.enter_context(tc.tile_pool(name="psD", bufs=1, space="PSUM"))

    def ap(t, offset_extra, dims):
        base = t[:] if not isinstance(t, bass.AP) else t
        return bass.AP(tensor=base.tensor, offset=base.offset + offset_extra,
                       ap=dims)

    # ---- input loads (sync queue, in dependency order)
    BLOBA = consts.tile([NK, A_COLS], f32)
    nc.sync.dma_start(out=BLOBA, in_=ins["blobA"])
    BLOBM = consts.tile([NM, M_COLS], f32)
    nc.sync.dma_start(out=BLOBM, in_=ins["blobM"])
    BLOBS = consts.tile([NM, S_COLS], bf)
    nc.sync.dma_start(out=BLOBS, in_=ins["blobS"])
    BLOBX = consts.tile([C, X_COLS], bf)
    nc.sync.dma_start(out=BLOBX, in_=ins["blobX"])
    BLOBI = consts.tile([128, 128], f32)
    nc.sync.dma_start(out=BLOBI, in_=ins["blobI"])
    BLOBW = consts.tile([128, 384], bf)
    nc.sync.dma_start(out=BLOBW, in_=ins["blobW"])

    XW3 = BLOBA[:, A_XW:A_XW + 594].rearrange("p (a b) -> p a b", a=9)
    WOFF2 = BLOBA[:, A_WOFF:A_WOFF + 162].rearrange("p (a b) -> p a b", a=9)
    MCW = BLOBM[:, M_MCW:M_MCW + 432]
    MS = BLOBM[:, M_MS:M_MS + 6]
    BMOD = BLOBM[0:64, M_BMOD:M_BMOD + 1]
    SELI = BLOBS[:, S_SELI:S_SELI + 1152].rearrange("p (a b) -> p a b", a=9)
    SELW = BLOBS[:, S_SELW:S_SELW + 1152].rearrange("p (a b) -> p a b", a=9)
    WMODR = BLOBS[0:64, S_WMODR:S_WMODR + 576].rearrange(
        "p (a b) -> p a b", a=9)
    XMOD = BLOBX.rearrange("p (t s k) -> p t s k", t=9, s=6)
    IDENT = BLOBI
    WCNVA = BLOBW[:, 0:192].rearrange("p (a b) -> p a b", a=3)
    WCNVB = BLOBW[0:64, 192:384].rearrange("p (a b) -> p a b", a=3)

    # ---- early memsets (Pool)
    # FP[p, s, c]: k-contiguous feat per stream (k = 96*phi + j2), rows
    # 0:64 at col 1+k (left pad col 0), rows 64:128 at col k (the dx-fold
    # shift).  Cols past 99(+1) stay zero: the conv windows read them for
    # the phi=1 row, where feat is zero beyond k=98.
    FP = work.tile([128, NS, 200], bf)
    nc.gpsimd.memset(FP, 0.0)
    IDX16 = work.tile([128, 96], dt.int16)

    # ---- offset conv: 9 taps -> PSUM [66, 18] = IX || IY (pixel coords)
    ps_xy = psA.tile([NM, 18], f32, tag="ps_xy")
    for t in range(9):
        nc.tensor.matmul(
            ps_xy,
            lhsT=XW3[:, t, :],
            rhs=WOFF2[:, t, :],
            start=(t == 0),
            stop=(t == 8),
        )

    # ---- coordinate math (DVE): floor + clamps + bilinear weight products
    TI = work.tile([NM, 18], dt.int32)
    nc.vector.tensor_copy(TI, ps_xy)
    TF = work.tile([NM, 18], f32)
    nc.vector.tensor_copy(TF, TI)
    GT = work.tile([NM, 18], f32)
    nc.vector.tensor_tensor(GT, TF, ps_xy, Alu.is_gt)
    I0 = work.tile([NM, 18], f32)
    nc.vector.tensor_sub(I0, TF, GT)
    FR = work.tile([NM, 18], f32)
    nc.vector.tensor_sub(FR, ps_xy, I0)

    # V = (Y0P, Y1P, XP) clipped+1.  The x98 row coordinate comes from the
    # cols 9:18 group (base j + DIRX), the pair/column one from cols 0:9 —
    # this matches the reference's swapped-axes grid_sample (as in v1).
    # Pool can't read PSUM, so V reads the SBUF I0 tile.  XP goes on DVE so
    # both engines finish V at about the same time; the idx path (V -> RHSI
    # -> scatter matmuls) is the critical chain, P/RHSW come after.
    V = work.tile([NM, 3, ND], f32)
    nc.gpsimd.tensor_scalar(V[:, 0, :], I0[:, 9:18], 1.0, 0.0, Alu.add,
                            Alu.max)
    nc.gpsimd.tensor_scalar(V[:, 0, :], V[:, 0, :], 97.0, None, Alu.min)
    nc.gpsimd.tensor_scalar(V[:, 1, :], I0[:, 9:18], 2.0, 0.0, Alu.add,
                            Alu.max)
    nc.gpsimd.tensor_scalar(V[:, 1, :], V[:, 1, :], 97.0, None, Alu.min)
    nc.vector.tensor_scalar(V[:, 2, :], I0[:, 0:9], 1.0, 0.0, Alu.add,
                            Alu.max)
    nc.vector.tensor_scalar(V[:, 2, :], V[:, 2, :], 97.0, None, Alu.min)

    # ---- idx scatter operands first (critical path)
    RHSI = work.tile([NM, ND, 3, NS, 8], bf)
    for d in range(9):
        dst = RHSI[:, d]
        src_m = ap(BLOBM, M_MCW + 48 * d,
                   [BLOBM[:].ap[0], [0, 3], [8, NS], [1, 8]])
        src_v = ap(V, d, [V[:].ap[0], [ND, 3], [0, NS], [0, 8]])
        eng = nc.gpsimd if d < 2 else nc.vector
        eng.tensor_tensor(dst, src_m, src_v, Alu.mult)

    # P = (w00, w01, w10, w11) corner weight products (DVE).  INBX zeroes
    # both x-corners when x0 < -1 (x1 would otherwise read a real pixel
    # through the clamped pad column).
    FX = FR[:, 0:9]
    FY = FR[:, 9:18]
    INBX = work.tile([NM, ND], f32)
    nc.vector.tensor_scalar(INBX, I0[:, 0:9], -1.0, None, Alu.is_ge)
    A1 = work.tile([NM, ND], f32)
    nc.vector.tensor_scalar(A1, FX, -1.0, 1.0, Alu.mult, Alu.add)
    nc.vector.tensor_mul(A1, A1, INBX)
    FX2 = work.tile([NM, ND], f32)
    nc.vector.tensor_mul(FX2, FX, INBX)
    B1 = work.tile([NM, ND], f32)
    nc.vector.tensor_scalar(B1, FY, -1.0, 1.0, Alu.mult, Alu.add)
    P = work.tile([NM, 4, ND], f32)
    nc.vector.tensor_mul(P[:, 0, :], B1, A1)
    nc.vector.tensor_mul(P[:, 1, :], B1, FX2)
    nc.vector.tensor_mul(P[:, 2, :], FY, A1)
    nc.vector.tensor_mul(P[:, 3, :], FY, FX2)

    # ---- scatter matmuls: idx [16, 3, 48] and W4S [128, 24]
    ps_yx = psA.tile([128, 3, NS, 8], f32, tag="ps_xy")
    for d in range(9):
        nc.tensor.matmul(ps_yx, lhsT=SELI[:, d, :], rhs=RHSI[:, d],
                         start=(d == 0), stop=(d == 8))
    # idx = 98*(y+1) + (x+1), int16, 16-wrapped.  high_priority keeps the
    # scheduler from slipping the W4S copy ahead of these on the DVE.
    # Only one op input may read PSUM -> copy ps_yx to SBUF first.
    with tc.high_priority():
        YX = work.tile([128, 3, NS, 8], f32)
        nc.vector.tensor_copy(YX, ps_yx)
        ix = IDX16[:]
        nc.vector.scalar_tensor_tensor(
            bass.AP(tensor=ix.tensor, offset=ix.offset,
                    ap=[ix.ap[0], [16, NS], [1, 8]]),
            YX[:, 0], 98.0, YX[:, 2], Alu.mult, Alu.add)
        nc.vector.scalar_tensor_tensor(
            bass.AP(tensor=ix.tensor, offset=ix.offset + 8,
                    ap=[ix.ap[0], [16, NS], [1, 8]]),
            YX[:, 1], 98.0, YX[:, 2], Alu.mult, Alu.add)

    # ---- gather, split in two stream-halves (chunk cc = 2s + ybank) so
    # the first three streams' combine overlaps the second transfer
    xh_src = bass.AP(tensor=xh.tensor, offset=xh.offset,
                     ap=[[64, 9604], [1, 128]])
    VV = work.tile([128, NCH, 128], f32)
    nc.gpsimd.dma_gather(out_ap=VV[:, 0:6, :], in_ap=xh_src,
                         idxs_ap=IDX16[:, 0:48],
                         num_idxs=768, num_idxs_reg=768,
                         elem_size=128, elem_step=64,
                         single_packet=False)
    nc.gpsimd.dma_gather(out_ap=VV[:, 6:12, :], in_ap=xh_src,
                         idxs_ap=IDX16[:, 48:96],
                         num_idxs=768, num_idxs_reg=768,
                         elem_size=128, elem_step=64,
                         single_packet=False)

    RHSW = work.tile([NM, ND, NS, 4], bf)
    for d in range(9):
        dst = RHSW[:, d]
        src_m = ap(BLOBM, M_MS, [BLOBM[:].ap[0], [1, NS], [0, 4]])
        src_p = ap(P, d, [P[:].ap[0], [0, NS], [ND, 4]])
        nc.gpsimd.tensor_tensor(dst, src_m, src_p, Alu.mult)

    ps_w = psA.tile([128, NS, 4], f32, tag="ps_xy")
    for d in range(9):
        nc.tensor.matmul(ps_w, lhsT=SELW[:, d, :], rhs=RHSW[:, d],
                         start=(d == 0), stop=(d == 8))
    W4S = work.tile([128, NS, 4], f32)
    nc.scalar.copy(W4S, ps_w)

    # ---- modulation conv (PE, output replicated over 64 partitions)
    MODA = work.tile([C, NS, 99], f32)
    for g in range(2):
        ps_m = psM.tile([C, 3, 99], f32, tag=f"ps_m{g}")
        for t in range(9):
            nc.tensor.matmul(
                ps_m,
                lhsT=WMODR[:, t, :],
                rhs=XMOD[:, t, 3 * g:3 * g + 3, :],
                start=(t == 0),
                stop=(t == 8),
            )
        nc.scalar.activation(MODA[:, 3 * g:3 * g + 3, :], ps_m,
                             Act.Sigmoid, bias=BMOD, scale=1.0)

    # ---- fold mod into the slot weights: transpose MODA per stream (PE,
    # during the gather window) and multiply the [99]-slot column into W4S.
    for s in range(NS):
        ps_mt = psM.tile([99, C], f32, tag=f"ps_m{s % 2}")
        nc.tensor.transpose(ps_mt, MODA[:, s, :], IDENT[0:64, 0:64])
        pm = ps_mt[:]
        nc.vector.tensor_tensor(
            W4S[0:99, s, :], W4S[0:99, s, :],
            bass.AP(tensor=pm.tensor, offset=pm.offset,
                    ap=[[pm.ap[0][0], 99], [0, 4]]),
            Alu.mult)

    # ---- combine + transpose + feat (per stream)
    S6 = work.tile([128, NS, 128], f32)
    TA0 = work.tile([128, C], f32)
    TB0 = work.tile([128, C], f32)
    TA1 = work.tile([128, C], f32)
    TB1 = work.tile([128, C], f32)
    TAs, TBs = [TA0, TA1], [TB0, TB1]
    for s in range(NS):
        eng = nc.vector
        TA, TB = TAs[s % 2], TBs[s % 2]
        eng.tensor_scalar(TA, VV[:, 2 * s, 0:64], W4S[:, s, 0:1], None,
                          Alu.mult)
        eng.scalar_tensor_tensor(TB, VV[:, 2 * s, 64:128], W4S[:, s, 1:2],
                                 TA, Alu.mult, Alu.add)
        eng.scalar_tensor_tensor(TA, VV[:, 2 * s + 1, 0:64], W4S[:, s, 2:3],
                                 TB, Alu.mult, Alu.add)
        eng.scalar_tensor_tensor(S6[:, s, 0:64], VV[:, 2 * s + 1, 64:128],
                                 W4S[:, s, 3:4], TA, Alu.mult, Alu.add)
        # duplicate cols (Pool, off the DVE path) so ONE transpose fills
        # both partition halves of ps_t with feat
        nc.gpsimd.tensor_copy(S6[:, s, 64:128], S6[:, s, 0:64])
        ps_t = psC.tile([128, 128], f32, tag=f"ps_t{s % 2}")
        nc.tensor.transpose(ps_t, S6[:, s, :], IDENT)
        nc.scalar.copy(FP[0:64, s, 1:100], ps_t[0:64, 0:99])
        nc.scalar.copy(FP[64:128, s, 0:99], ps_t[64:128, 0:99])

    # ---- final conv strips.  dx taps 0,1 are folded into a 128-wide
    # contraction (group A, shifted feat rows 64:128); dx=2 is group B.
    # dy order +1 (rows [0:2], start), -1 (rows [2:4], start — disjoint),
    # 0 (accumulate) makes every psum row's first write a start.
    for s in range(NS):
        ps_c = psD.tile([C, 4, 96], f32, tag=f"ps_c{s % 3}")
        # start only the first matmul: it marks the whole psum zero-region
        # pending-zero, so each later window's first touch zero-fills itself
        fa = FP[:, s]
        fb = FP[0:64, s]
        rhsA = bass.AP(tensor=fa.tensor, offset=fa.offset,
                       ap=[fa.ap[0], [96, 2], [1, 96]])
        rhsB = bass.AP(tensor=fb.tensor, offset=fb.offset + 2,
                       ap=[fb.ap[0], [96, 2], [1, 96]])
        for n, dyk in enumerate((2, 0, 1)):
            nc.tensor.matmul(
                ps_c[:, 2 - dyk:4 - dyk, :],
                lhsT=WCNVA[:, dyk, :],
                rhs=rhsA,
                start=(n == 0),
                stop=False,
                skip_group_check=True,
            )
        for n, dyk in enumerate((2, 0, 1)):
            nc.tensor.matmul(
                ps_c[:, 2 - dyk:4 - dyk, :],
                lhsT=WCNVB[:, dyk, :],
                rhs=rhsB,
                start=False,
                stop=(n == 2),
                skip_group_check=True,
            )
        if s % 2 == 0:
            OUTS = loop_sb.tile([C, 2, 4, 96], f32, tag=f"outs{(s // 2) % 2}",
                                name=f"OUTS{s // 2}")
            nc.scalar.copy(OUTS[:, 0], ps_c)
        else:
            nc.vector.tensor_copy(OUTS[:, 1], ps_c)
            nc.sync.dma_start(out=strips_out[:, s - 1:s + 1], in_=OUTS)

    ctx.close()


@functools.lru_cache(maxsize=1)
def _build_program():
    from contextlib import ExitStack

    import concourse.bacc as bacc
    import concourse.tile as tile
    from concourse import mybir

    dt = mybir.dt
    nc = bacc.Bacc("TRN2", target_bir_lowering=False, debug=False)
    ins = {
        "xh": nc.dram_tensor("xh", [XHROWS, C], dt.float32,
                             kind="ExternalInput").ap(),
        "blobA": nc.dram_tensor("blobA", [NK, A_COLS], dt.float32,
                                kind="ExternalInput").ap(),
        "blobM": nc.dram_tensor("blobM", [NM, M_COLS], dt.float32,
                                kind="ExternalInput").ap(),
        "blobS": nc.dram_tensor("blobS", [NM, S_COLS], dt.bfloat16,
                                kind="ExternalInput").ap(),
        "blobX": nc.dram_tensor("blobX", [C, X_COLS], dt.bfloat16,
                                kind="ExternalInput").ap(),
        "blobI": nc.dram_tensor("blobI", [128, 128], dt.float32,
                                kind="ExternalInput").ap(),
        "blobW": nc.dram_tensor("blobW", [128, 384], dt.bfloat16,
                                kind="ExternalInput").ap(),
    }
    outs = {
        "strips_out": nc.dram_tensor("strips_out", [C, NS, 4, 96],
                                     dt.float32, kind="ExternalOutput").ap(),
    }
    with ExitStack() as ctx:
        tc = ctx.enter_context(tile.TileContext(nc))
        emit_kernel(tc, outs, ins)
    nc.compile()
    return nc


def _host_inputs(inputs):
    arrs = {k: np.asarray(v, np.float32) for k, v in inputs.items()}
    in_maps = []
    for core in range(8):
        b, part = core // 2, core % 2
        in_maps.append(_make_core_inputs(
            arrs["x"], arrs["w_off1"], arrs["b_off1"], arrs["w_off2"],
            arrs["b_off2"], arrs["w_mod"], arrs["b_mod"],
            arrs["conv_weight"], float(arrs["alpha"][0]), b, part))
    return in_maps


def _assemble(results):
    out = np.zeros((4, C, H, W), np.float32)
    for core, res in enumerate(results):
        b, part = core // 2, core % 2
        i0 = 6 * part
        strips = res["strips_out"]
        for s in range(NS):
            r0 = 9 * (i0 + s) - 1
            if r0 < 0:
                out[b][:, 0:r0 + 4, :] = strips[:, s, -r0:, :]
            elif r0 + 4 <= H:
                out[b][:, r0:r0 + 4, :] = strips[:, s]
    return out


def kernel(**inputs) -> np.ndarray:
    from concourse.bass_utils import run_bass_kernel_spmd

    nc = _build_program()
    in_maps = _host_inputs(inputs)
    res = run_bass_kernel_spmd(nc, in_maps, core_ids=list(range(8)))
    return _assemble(res.results)


if __name__ == "__main__":
    d = dict(np.load("/root/problem/inputs_cache.npz"))
    out = kernel(**d)
    ref = np.load("/root/problem/expected_np.npy")
    err = np.abs(out - ref).max()
    print("absmax err:", err, "rel:", err / np.abs(ref).max())
